# revision 22
# baseline (speedup 1.0000x reference)
"""MetacognitionModule (MoE routing) Trainium2 kernel.

Sharding: data-parallel over batch — core i handles batch i (B=8, 8 cores).
Everything is local per core: the router (mean-pool -> 3-layer MLP -> double
softmax) and all 8 expert MLPs run on the core that owns the batch, so no
collectives are needed.

Per-core dataflow (S=2048 tokens, H=2048, Hh=1024, E=8 experts):
  - All 4 chunks' xT tiles (DMA-transposed) are loaded up front on the Sync
    HWDGE queue and stay resident (64 KiB/partition) — x is read from HBM
    exactly once.
  - PE warm-up: a run of dummy matmuls on memset tiles heads the PE stream so
    the HAM clock-gate reaches 8/8 while the first weights/x DMAs land.
  - Router: pooled = mean_s x via one DVE free-dim reduce per chunk (no PE,
    no extra x loads). The tiny router MLP + softmaxes are emitted in the PE
    stream between expert 1's L1 and L2, by which point pooled and the wm
    DMAs are long done. wbc holds [w0, w1/w0 .. w7/w0, w7] so expert 0's
    combine needs no router output at all.
  - Experts, chunked over S (4 chunks of 512 tokens), expert-inner,
    weights streamed per (chunk, expert):
      L1: heT[f,s] = relu(W1[e].T @ xT + b1)   (bias via ACT per-partition)
      L2: z[s,h]  = heT.T @ W2[e]
      e=0   : acc       = tanh(z)              (ACT straight to SBUF acc)
      e=1..6: acc      += (w_e/w_0) * tanh(z)  (ACT tanh + DVE fused mul-add)
      after e6: acc    *= w_0                  (off critical path)
      e=7   : out_bf16  = w_7 * tanh(z) + acc  (DVE writes bf16 directly)
  - out tiles stored per s-subtile on the Scalar HWDGE queue as they finish,
    in natural [S,H] layout, bf16 (host casts back to f32).
All expert matmuls bf16 with fp32 PSUM accumulation.
"""

import sys

for _p in ("/opt/trn_rl_repo", "/root/.axon_site/_ro/trn_rl_repo"):
    if _p not in sys.path:
        sys.path.insert(0, _p)

import ml_dtypes
import numpy as np

import concourse.bacc as bacc
import concourse.bass as bass
import concourse.mybir as mybir
import concourse.tile as tile
from concourse.bass_utils import run_bass_kernel_spmd

BF16 = ml_dtypes.bfloat16
F32 = mybir.dt.float32
BF = mybir.dt.bfloat16
AF = mybir.ActivationFunctionType
ALU = mybir.AluOpType

B, S, H, M, E = 8, 2048, 2048, 256, 8
Hh = H // 2
CHUNK = 512
NCHUNK = S // CHUNK          # 4
NST = CHUNK // 128           # 4 s-subtiles per chunk
NHT = H // 512               # 4 output h tiles (512 wide)
NFT = Hh // 128              # 8 L1 output f tiles
NKH = H // 128               # 16 k tiles over h
NDUMMY = 36                  # PE warm-up matmuls (~10us of coverage)

_NC = {}


def _softmax_1x8(nc, pool, vec, out, tagp):
    """vec, out: [1, E] f32 sbuf APs. out = softmax(vec) along free dim.
    No max-subtraction: inputs here are probabilities or ~1e-3 logits, so
    exp() is always in range."""
    t = pool.tile([1, E], F32, tag=tagp + "t", name=tagp + "t")
    nc.scalar.activation(t[:], vec, AF.Exp)
    sm = pool.tile([1, 1], F32, tag=tagp + "sm", name=tagp + "sm")
    nc.vector.tensor_reduce(sm[:], t[:], mybir.AxisListType.X, ALU.add)
    rs = pool.tile([1, 1], F32, tag=tagp + "rs", name=tagp + "rs")
    nc.vector.reciprocal(rs[:], sm[:])
    nc.vector.tensor_scalar(out, t[:], rs[0:1, 0:1], None, ALU.mult)


def build(with_bias1=False, with_bias2=False):
    nc = bacc.Bacc("TRN2", target_bir_lowering=False, debug=False, num_devices=B)

    # x arrives host-pretransposed: [ck, p, ht, c] so each chunk's xT tile is
    # ONE contiguous 2MiB full-rate DMA (HWDGE completion latency amortized).
    x_d = nc.dram_tensor("x", [NCHUNK, 128, NKH, CHUNK], BF, kind="ExternalInput")
    # W1/W2 arrive host-preshuffled to SBUF layout:
    # W1: [E, half, p, kt, f]  (host-preshuffled, halves of h-contraction)
    # W2: [E, p, ht, fk, c]    (host-preshuffled, ht-major)
    w1_d = nc.dram_tensor("W1", [E, 2, 128, 8, Hh], BF, kind="ExternalInput")
    w2_d = nc.dram_tensor("W2", [E, 128, 4, NFT, 512], BF, kind="ExternalInput")
    b1_d = nc.dram_tensor("b1", [E, Hh], F32, kind="ExternalInput")
    b2_d = nc.dram_tensor("b2", [E, H], BF, kind="ExternalInput")
    wm1_d = nc.dram_tensor("Wm1", [128, NKH * M], BF, kind="ExternalInput")
    bm1_d = nc.dram_tensor("bm1", [M], F32, kind="ExternalInput")
    wm2_d = nc.dram_tensor("Wm2", [128, 2 * M], BF, kind="ExternalInput")
    bm2_d = nc.dram_tensor("bm2", [M], F32, kind="ExternalInput")
    wm3_d = nc.dram_tensor("Wm3", [128, 2 * E], BF, kind="ExternalInput")
    bm3_d = nc.dram_tensor("bm3", [E], F32, kind="ExternalInput")
    eff_d = nc.dram_tensor("eff", [E], F32, kind="ExternalInput")
    out_d = nc.dram_tensor("out", [S, H], BF, kind="ExternalOutput")

    with tile.TileContext(nc) as tc:
        with (
            tc.tile_pool(name="persist", bufs=1) as pp,
            tc.tile_pool(name="router", bufs=1) as rp,
            tc.tile_pool(name="xt", bufs=1) as xtp,
            tc.tile_pool(name="w1", bufs=1) as w1p,
            tc.tile_pool(name="w2", bufs=1) as w2p,
            tc.tile_pool(name="bias", bufs=2) as bp,
            tc.tile_pool(name="he", bufs=2) as hep,
            tc.tile_pool(name="acc", bufs=1) as accp,
            tc.tile_pool(name="ye", bufs=2) as yep,
            tc.tile_pool(name="outb", bufs=2) as outp,
            tc.tile_pool(name="ps1", bufs=2, space=bass.MemorySpace.PSUM) as ps1p,
            tc.tile_pool(name="ps2", bufs=2, space=bass.MemorySpace.PSUM) as ps2p,
        ):
            # wbc layout: col 0 = w0, cols 1..7 = w_e/w_0, col 8 = w_7
            wbc = pp.tile([128, E + 1], F32)
            ones_bf = pp.tile([1, 128], BF)    # ones row for bias2 matmuls
            nc.vector.memset(ones_bf[:], 1.0)
            pooled_f = pp.tile([128, NKH], F32)
            nc.vector.memset(pooled_f[:], 0.0)

            # ---- PE warm-up: dummy matmuls on zeroed tiles ----
            dum_w = pp.tile([128, 128], BF)
            dum_x = pp.tile([128, 512], BF)
            nc.vector.memset(dum_w[:], 0.0)
            nc.vector.memset(dum_x[:], 0.0)
            dum_ps = ps1p.tile([128, 2, CHUNK], F32, tag="ps1", name="dum_ps")
            for i in range(NDUMMY):
                nc.tensor.matmul(
                    dum_ps[:, 0, :], dum_w[:], dum_x[:],
                    start=True, stop=True, skip_group_check=True,
                )

            # ---- all of x, transposed, resident for the whole kernel ----
            # ck0 rides the otherwise-idle Sync HWDGE queue in parallel with
            # expert 0's weights; ck1-3 (not needed until ~50us) queue on the
            # SWDGE FIFO *behind* W1/W2-e0 so they don't steal early HBM BW.
            xt_all = []
            for ck in range(NCHUNK):
                xt = xtp.tile([128, NKH, CHUNK], BF, tag=f"xt{ck}", name=f"xt{ck}")
                xt_all.append(xt)

            def load_w1(ck, e, engines=None):
                halves = []
                for half in range(2):
                    t = w1p.tile([128, 8, Hh], BF, tag=f"w1{half}",
                                 name=f"w1_{ck}_{e}_{half}")
                    eng = engines[half] if engines else nc.sync
                    eng.dma_start(t[:], w1_d[e, half])
                    halves.append(t)
                return halves

            def load_w2(ck, e):
                w2 = w2p.tile([128, 4, NFT, 512], BF, tag="w2", name=f"w2_{ck}_{e}")
                nc.sync.dma_start(w2[:], w2_d[e])
                return w2

            def load_b(ck, e):
                b1t = None
                if with_bias1:
                    b1t = bp.tile([128, NFT], F32, tag="b1", name=f"b1_{ck}_{e}")
                    nc.sync.dma_start(
                        b1t[:], b1_d[e].rearrange("(t p) -> p t", p=128)
                    )
                b2t = None
                if with_bias2:
                    b2t = bp.tile([1, H], BF, tag="b2", name=f"b2_{ck}_{e}")
                    nc.sync.dma_start(b2t[:], b2_d[e:e + 1, :])
                return b1t, b2t

            # Bulk traffic rides the Sync HWDGE FIFO (ramps ~4us earlier than
            # SWDGE and needs no Q7 descriptor generation); the small strided
            # router-weight loads stay on the SWDGE queue out of the way.
            with tc.high_priority():
                # Two HWDGE rings in parallel (transfers serialize per ring):
                #   sync:   W1-e0 kt0-7, W2-e0, x chunks 1-3
                #   scalar: x chunk 0,   W1-e0 kt8-15
                # -> the two-pass L1 starts ~15us in, right off the dummies.
                nc.scalar.dma_start(xt_all[0][:], x_d[0])
                w1h0 = load_w1(0, 0, engines=(nc.sync, nc.scalar))
                b0 = load_b(0, 0)
                preload = {(0, 0): (w1h0, load_w2(0, 0), b0)}
                for ck in range(1, NCHUNK):
                    nc.sync.dma_start(xt_all[ck][:], x_d[ck])
                wm1 = rp.tile([128, NKH, M], BF)
                nc.gpsimd.dma_start(wm1[:], wm1_d[:].rearrange("p (t f) -> p t f", f=M))
                bm1 = rp.tile([128, 2], F32)
                nc.gpsimd.dma_start(bm1[:], bm1_d[:].rearrange("(t p) -> p t", p=128))
                wm2 = rp.tile([128, 2, M], BF)
                nc.gpsimd.dma_start(wm2[:], wm2_d[:].rearrange("p (t f) -> p t f", f=M))
                bm2 = rp.tile([128, 2], F32)
                nc.gpsimd.dma_start(bm2[:], bm2_d[:].rearrange("(t p) -> p t", p=128))
                wm3 = rp.tile([128, 2, E], BF)
                nc.gpsimd.dma_start(wm3[:], wm3_d[:].rearrange("p (t f) -> p t f", f=E))
                bm3 = rp.tile([1, E], F32)
                nc.gpsimd.dma_start(bm3[:], bm3_d[:].rearrange("(a e) -> a e", a=1))
                eff = rp.tile([1, E], F32)
                nc.gpsimd.dma_start(eff[:], eff_d[:].rearrange("(a e) -> a e", a=1))

            # ---- router pooling: one DVE free-dim reduce per chunk ----
            for ck in range(NCHUNK):
                ptmp = rp.tile([128, NKH], F32, tag="ptmp", name=f"ptmp{ck}")
                nc.vector.tensor_reduce(
                    ptmp[:], xt_all[ck][:, :, :], mybir.AxisListType.X, ALU.add
                )
                nc.vector.tensor_tensor(
                    pooled_f[:], pooled_f[:], ptmp[:], ALU.add
                )

            rst = {}

            def emit_router_mlp():
                """pooled_f -> router MLP -> double softmax -> rrow
                [w0, r1..r7, w7]. Emitted between e0 and e1: the serial
                DVE/ACT softmax chain overlaps e1's L1 matmuls."""
                pooled = rp.tile([128, NKH], BF)
                nc.vector.tensor_scalar(pooled[:], pooled_f[:], 1.0 / S, None, ALU.mult)
                ones_f = rp.tile([1, 128], F32)
                nc.vector.memset(ones_f[:], 1.0)
                ones_b1 = rp.tile([1, 1], BF)
                nc.vector.memset(ones_b1[:], 1.0)

                h1t = rp.tile([128, 2], BF)
                for ft in range(2):
                    ps = ps2p.tile([128, E + 1], F32, tag="ps2", name=f"rps1_{ft}")
                    for kt in range(NKH):
                        nc.tensor.matmul(
                            ps[:, 0:1],
                            wm1[:, kt, ft * 128:(ft + 1) * 128],
                            pooled[:, kt:kt + 1],
                            start=(kt == 0), stop=(kt == NKH - 1),
                        )
                    nc.vector.tensor_scalar(
                        h1t[:, ft:ft + 1], ps[:, 0:1], bm1[:, ft:ft + 1], 0.0,
                        ALU.add, ALU.max,
                    )
                h2t = rp.tile([128, 2], BF)
                for ft in range(2):
                    ps = ps2p.tile([128, E + 1], F32, tag="ps2", name=f"rps2_{ft}")
                    for kt in range(2):
                        nc.tensor.matmul(
                            ps[:, 0:1],
                            wm2[:, kt, ft * 128:(ft + 1) * 128],
                            h1t[:, kt:kt + 1],
                            start=(kt == 0), stop=(kt == 1),
                        )
                    nc.vector.tensor_scalar(
                        h2t[:, ft:ft + 1], ps[:, 0:1], bm2[:, ft:ft + 1], 0.0,
                        ALU.add, ALU.max,
                    )
                bm3b = rp.tile([1, E], BF)
                nc.vector.tensor_copy(bm3b[:], bm3[:])
                psl = ps2p.tile([128, E + 1], F32, tag="ps2", name="rpsl")
                for kt in range(2):
                    nc.tensor.matmul(
                        psl[0:1, 0:E], h2t[:, kt:kt + 1], wm3[:, kt, :],
                        start=(kt == 0), stop=False,
                    )
                nc.tensor.matmul(
                    psl[0:1, 0:E], ones_b1[0:1, 0:1], bm3b[0:1, :],
                    start=False, stop=True,
                )
                logits = rp.tile([1, E], F32)
                nc.vector.tensor_copy(logits[:], psl[0:1, 0:E])

                probs = rp.tile([1, E], F32)
                _softmax_1x8(nc, rp, logits[:], probs[:], "sm1")
                wpre = rp.tile([1, E], F32)
                nc.vector.tensor_tensor(wpre[:], probs[:], eff[:], ALU.mult)
                wrow = rp.tile([1, E], F32)
                _softmax_1x8(nc, rp, wpre[:], wrow[:], "sm2")

                # rrow = [w0, w1/w0, ..., w7/w0, w7]
                rcp0 = rp.tile([1, 1], F32)
                nc.vector.reciprocal(rcp0[:], wrow[0:1, 0:1])
                rrow = rp.tile([1, E + 1], F32)
                nc.vector.tensor_scalar(
                    rrow[:, 0:E], wrow[:], rcp0[0:1, 0:1], None, ALU.mult
                )
                nc.vector.tensor_copy(rrow[:, 0:1], wrow[0:1, 0:1])
                nc.vector.tensor_copy(rrow[:, E:E + 1], wrow[0:1, E - 1:E])

                rst["rrow"] = rrow
                rst["ones_f"] = ones_f

            def emit_router_bcast():
                """Broadcast rrow to all 128 partitions. Emitted after e1's
                L1 so the chain behind rrow is long done."""
                psw = ps2p.tile([128, E + 1], F32, tag="ps2", name="rpsw")
                nc.tensor.matmul(
                    psw[:], rst["ones_f"][0:1, :], rst["rrow"][0:1, :],
                    start=True, stop=True,
                )
                nc.vector.tensor_copy(wbc[:], psw[:])

            # ---------------- experts ----------------
            for ck in range(NCHUNK):
                xt = xt_all[ck]
                acc_tiles = [
                    accp.tile([128, H], F32, tag=f"acc{st}", name=f"acc{ck}_{st}")
                    for st in range(NST)
                ]
                out_tiles = [None] * NST
                for e in range(E):
                    if ck == 0 and e == 1:
                        emit_router_mlp()
                    if (ck, e) in preload:
                        w1h, w2, (b1t, b2t) = preload[(ck, e)]
                    else:
                        w1h = load_w1(ck, e)
                        w2 = load_w2(ck, e)
                        b1t, b2t = load_b(ck, e)

                    he = hep.tile([128, NFT, CHUNK], BF, tag="he", name=f"he_{ck}_{e}")
                    if ck == 0 and e == 0:
                        # startup special: contract kt 0-7 (first W1 half to
                        # arrive) across all 4 wide tiles, then kt 8-15 —
                        # PE starts ~6us earlier than waiting for full W1.
                        ps_sp = []
                        for fp in range(NFT // 2):
                            pool = ps1p if fp < 2 else ps2p
                            tag = "ps1" if fp < 2 else "ps2"
                            ps_sp.append(pool.tile(
                                [128, 2, CHUNK], F32, tag=tag, name=f"ps1sp_{fp}"
                            ))
                        for khalf in range(2):
                            for fp in range(NFT // 2):
                                for sub in range(2):
                                    ft = 2 * fp + sub
                                    for kt in range(khalf * 8, khalf * 8 + 8):
                                        nc.tensor.matmul(
                                            ps_sp[fp][:, sub, :],
                                            w1h[khalf][:, kt % 8,
                                                       ft * 128:(ft + 1) * 128],
                                            xt[:, kt, :],
                                            start=(kt == 0), stop=(kt == NKH - 1),
                                            skip_group_check=True,
                                        )
                        for fp in range(NFT // 2):
                            if with_bias1:
                                for sub in range(2):
                                    ft = 2 * fp + sub
                                    nc.scalar.activation(
                                        he[:, ft, :], ps_sp[fp][:, sub, :],
                                        AF.Relu, bias=b1t[:, ft:ft + 1],
                                    )
                            else:
                                nc.scalar.activation(
                                    he[:, 2 * fp:2 * fp + 2, :], ps_sp[fp][:], AF.Relu
                                )
                    for fp in range(NFT // 2):
                        if ck == 0 and e == 0:
                            break
                        ps = ps1p.tile([128, 2, CHUNK], F32, tag="ps1",
                                       name=f"ps1_{ck}_{e}_{fp}")
                        for sub in range(2):
                            ft = 2 * fp + sub
                            for kt in range(NKH):
                                nc.tensor.matmul(
                                    ps[:, sub, :],
                                    w1h[kt // 8][:, kt % 8, ft * 128:(ft + 1) * 128],
                                    xt[:, kt, :],
                                    start=(kt == 0), stop=(kt == NKH - 1),
                                )
                        if with_bias1:
                            for sub in range(2):
                                ft = 2 * fp + sub
                                nc.scalar.activation(
                                    he[:, ft, :], ps[:, sub, :], AF.Relu,
                                    bias=b1t[:, ft:ft + 1],
                                )
                        else:
                            nc.scalar.activation(
                                he[:, 2 * fp:2 * fp + 2, :], ps[:], AF.Relu
                            )

                    if ck == 0 and e == 1:
                        emit_router_bcast()

                    for st in range(NST):
                        if e == E - 1 and out_tiles[st] is None:
                            out_tiles[st] = outp.tile(
                                [128, H], BF, tag="outt", name=f"out_{ck}_{st}"
                            )
                        for hp in range(NHT // 2):
                            ps2 = ps2p.tile([128, 2, 512], F32, tag="ps2",
                                            name=f"ps2_{ck}_{e}_{st}_{hp}")
                            for sub in range(2):
                                ht = 2 * hp + sub
                                for fk in range(NFT):
                                    nc.tensor.matmul(
                                        ps2[:, sub, :],
                                        he[:, fk, st * 128:(st + 1) * 128],
                                        w2[:, ht, fk, :],
                                        start=(fk == 0),
                                        stop=(not with_bias2 and fk == NFT - 1),
                                    )
                                if with_bias2:
                                    nc.tensor.matmul(
                                        ps2[:, sub, :], ones_bf[0:1, :],
                                        b2t[0:1, ht * 512:(ht + 1) * 512],
                                        start=False, stop=True,
                                    )
                            accs = acc_tiles[st][:, hp * 1024:(hp + 1) * 1024]
                            if e == 0:
                                # no router dependency: plain tanh into acc
                                nc.scalar.activation(accs, ps2[:], AF.Tanh)
                            else:
                                ye = yep.tile([128, 1024], F32, tag="ye",
                                              name=f"ye_{ck}_{e}_{st}_{hp}")
                                nc.scalar.activation(ye[:], ps2[:], AF.Tanh)
                                if e < E - 1:
                                    nc.vector.scalar_tensor_tensor(
                                        accs, ye[:], wbc[:, e:e + 1], accs,
                                        ALU.mult, ALU.add,
                                    )
                                    if e == E - 2:
                                        # pre-scale by w0 off the critical path
                                        nc.vector.tensor_scalar(
                                            accs, accs, wbc[:, 0:1], None, ALU.mult
                                        )
                                else:
                                    # final: out = w7*tanh(z) + acc  (bf16)
                                    nc.vector.scalar_tensor_tensor(
                                        out_tiles[st][:, hp * 1024:(hp + 1) * 1024],
                                        ye[:], wbc[:, E:E + 1], accs,
                                        ALU.mult, ALU.add,
                                    )
                                    r0 = ck * CHUNK + st * 128
                                    nc.sync.dma_start(
                                        out_d[r0:r0 + 128,
                                              hp * 1024:(hp + 1) * 1024],
                                        out_tiles[st][:, hp * 1024:(hp + 1) * 1024],
                                    )

    nc.compile()
    return nc


def _get_nc(with_bias1=False, with_bias2=False):
    key = (with_bias1, with_bias2)
    if key not in _NC:
        _NC[key] = build(with_bias1, with_bias2)
    return _NC[key]


def prep_in_maps(inputs):
    x = np.asarray(inputs["x"], np.float32)
    xbf = x.astype(BF16)
    # pre-transpose to [ht, ck, p, c] blocks (see build()): xT[h, s] blocked
    xts = [
        np.ascontiguousarray(
            xbf[b].T.reshape(NKH, 128, NCHUNK, CHUNK).transpose(2, 1, 0, 3)
        )
        for b in range(B)
    ]
    w1 = np.asarray(inputs["W1"], np.float32).astype(BF16)   # [E, H, Hh]
    w2 = np.asarray(inputs["W2"], np.float32).astype(BF16)   # [E, Hh, H]
    # shuffle to SBUF layout (see build()): halves x partition-major
    w1s = np.ascontiguousarray(
        w1.reshape(E, 2, 8, 128, Hh).transpose(0, 1, 3, 2, 4)
    )
    w2s = np.ascontiguousarray(
        w2.reshape(E, 8, 128, 4, 512).transpose(0, 2, 3, 1, 4)
    )
    wm1 = np.asarray(inputs["Wm1"], np.float32).astype(BF16)
    wm1s = np.ascontiguousarray(
        wm1.reshape(16, 128, M).transpose(1, 0, 2).reshape(128, 16 * M)
    )
    wm2 = np.asarray(inputs["Wm2"], np.float32).astype(BF16)
    wm2s = np.ascontiguousarray(
        wm2.reshape(2, 128, M).transpose(1, 0, 2).reshape(128, 2 * M)
    )
    wm3 = np.asarray(inputs["Wm3"], np.float32).astype(BF16)
    wm3s = np.ascontiguousarray(
        wm3.reshape(2, 128, E).transpose(1, 0, 2).reshape(128, 2 * E)
    )
    shared = {
        "W1": w1s,
        "W2": w2s,
        "b1": np.asarray(inputs["b1"], np.float32),
        "b2": np.asarray(inputs["b2"], np.float32).astype(BF16),
        "Wm1": wm1s,
        "bm1": np.asarray(inputs["bm1"], np.float32),
        "Wm2": wm2s,
        "bm2": np.asarray(inputs["bm2"], np.float32),
        "Wm3": wm3s,
        "bm3": np.asarray(inputs["bm3"], np.float32),
        "eff": np.asarray(inputs["eff"], np.float32),
    }
    return [dict(shared, x=xts[b]) for b in range(B)]


def kernel(**inputs):
    wb1 = bool(np.any(np.asarray(inputs["b1"])))
    wb2 = bool(np.any(np.asarray(inputs["b2"])))
    nc = _get_nc(wb1, wb2)
    in_maps = prep_in_maps(inputs)
    res = run_bass_kernel_spmd(nc, in_maps, core_ids=list(range(B)))
    return np.stack([np.asarray(r["out"]).astype(np.float32) for r in res.results])


if __name__ == "__main__":
    rng = np.random.default_rng(0)
    s = 0.02
    ins = {
        "x": rng.standard_normal((B, S, H), dtype=np.float32),
        "Wm1": rng.standard_normal((H, M), dtype=np.float32) * s,
        "bm1": np.zeros(M, np.float32),
        "Wm2": rng.standard_normal((M, M), dtype=np.float32) * s,
        "bm2": np.zeros(M, np.float32),
        "Wm3": rng.standard_normal((M, E), dtype=np.float32) * s,
        "bm3": np.zeros(E, np.float32),
        "W1": rng.standard_normal((E, H, Hh), dtype=np.float32) * s,
        "b1": np.zeros((E, Hh), np.float32),
        "W2": rng.standard_normal((E, Hh, H), dtype=np.float32) * s,
        "b2": np.zeros((E, H), np.float32),
        "eff": np.ones(E, np.float32),
    }
    out = kernel(**ins)
    print("out", out.shape, out.dtype, float(np.abs(out).mean()))


# revision 23
# speedup vs baseline: 1.0028x; 1.0028x over previous
"""MetacognitionModule (MoE routing) Trainium2 kernel.

Sharding: data-parallel over batch — core i handles batch i (B=8, 8 cores).
Everything is local per core: the router (mean-pool -> 3-layer MLP -> double
softmax) and all 8 expert MLPs run on the core that owns the batch, so no
collectives are needed.

Per-core dataflow (S=2048 tokens, H=2048, Hh=1024, E=8 experts):
  - All 4 chunks' xT tiles (DMA-transposed) are loaded up front on the Sync
    HWDGE queue and stay resident (64 KiB/partition) — x is read from HBM
    exactly once.
  - PE warm-up: a run of dummy matmuls on memset tiles heads the PE stream so
    the HAM clock-gate reaches 8/8 while the first weights/x DMAs land.
  - Router: pooled = mean_s x via one DVE free-dim reduce per chunk (no PE,
    no extra x loads). The tiny router MLP + softmaxes are emitted in the PE
    stream between expert 1's L1 and L2, by which point pooled and the wm
    DMAs are long done. wbc holds [w0, w1/w0 .. w7/w0, w7] so expert 0's
    combine needs no router output at all.
  - Experts, chunked over S (4 chunks of 512 tokens), expert-inner,
    weights streamed per (chunk, expert):
      L1: heT[f,s] = relu(W1[e].T @ xT + b1)   (bias via ACT per-partition)
      L2: z[s,h]  = heT.T @ W2[e]
      e=0   : acc       = tanh(z)              (ACT straight to SBUF acc)
      e=1..6: acc      += (w_e/w_0) * tanh(z)  (ACT tanh + DVE fused mul-add)
      after e6: acc    *= w_0                  (off critical path)
      e=7   : out_bf16  = w_7 * tanh(z) + acc  (DVE writes bf16 directly)
  - out tiles stored per s-subtile on the Scalar HWDGE queue as they finish,
    in natural [S,H] layout, bf16 (host casts back to f32).
All expert matmuls bf16 with fp32 PSUM accumulation.
"""

import sys

for _p in ("/opt/trn_rl_repo", "/root/.axon_site/_ro/trn_rl_repo"):
    if _p not in sys.path:
        sys.path.insert(0, _p)

import ml_dtypes
import numpy as np

import concourse.bacc as bacc
import concourse.bass as bass
import concourse.mybir as mybir
import concourse.tile as tile
from concourse.bass_utils import run_bass_kernel_spmd

BF16 = ml_dtypes.bfloat16
F32 = mybir.dt.float32
BF = mybir.dt.bfloat16
AF = mybir.ActivationFunctionType
ALU = mybir.AluOpType

B, S, H, M, E = 8, 2048, 2048, 256, 8
Hh = H // 2
CHUNK = 512
NCHUNK = S // CHUNK          # 4
NST = CHUNK // 128           # 4 s-subtiles per chunk
NHT = H // 512               # 4 output h tiles (512 wide)
NFT = Hh // 128              # 8 L1 output f tiles
NKH = H // 128               # 16 k tiles over h
NDUMMY = 48                  # PE warm-up matmuls (~13us of coverage)

_NC = {}


def _softmax_1x8(nc, pool, vec, out, tagp):
    """vec, out: [1, E] f32 sbuf APs. out = softmax(vec) along free dim.
    No max-subtraction: inputs here are probabilities or ~1e-3 logits, so
    exp() is always in range."""
    t = pool.tile([1, E], F32, tag=tagp + "t", name=tagp + "t")
    nc.scalar.activation(t[:], vec, AF.Exp)
    sm = pool.tile([1, 1], F32, tag=tagp + "sm", name=tagp + "sm")
    nc.vector.tensor_reduce(sm[:], t[:], mybir.AxisListType.X, ALU.add)
    rs = pool.tile([1, 1], F32, tag=tagp + "rs", name=tagp + "rs")
    nc.vector.reciprocal(rs[:], sm[:])
    nc.vector.tensor_scalar(out, t[:], rs[0:1, 0:1], None, ALU.mult)


def build(with_bias1=False, with_bias2=False):
    nc = bacc.Bacc("TRN2", target_bir_lowering=False, debug=False, num_devices=B)

    # x arrives host-pretransposed: [ck, p, ht, c] so each chunk's xT tile is
    # ONE contiguous 2MiB full-rate DMA (HWDGE completion latency amortized).
    x_d = nc.dram_tensor("x", [NCHUNK, 128, NKH, CHUNK], BF, kind="ExternalInput")
    # W1/W2 arrive host-preshuffled to SBUF layout:
    # W1: [E, half, p, kt, f]  (host-preshuffled, halves of h-contraction)
    # W2: [E, p, ht, fk, c]    (host-preshuffled, ht-major)
    w1_d = nc.dram_tensor("W1", [E, 2, 128, 8, Hh], BF, kind="ExternalInput")
    w2_d = nc.dram_tensor("W2", [E, 128, 4, NFT, 512], BF, kind="ExternalInput")
    b1_d = nc.dram_tensor("b1", [E, Hh], F32, kind="ExternalInput")
    b2_d = nc.dram_tensor("b2", [E, H], BF, kind="ExternalInput")
    wm1_d = nc.dram_tensor("Wm1", [128, NKH * M], BF, kind="ExternalInput")
    bm1_d = nc.dram_tensor("bm1", [M], F32, kind="ExternalInput")
    wm2_d = nc.dram_tensor("Wm2", [128, 2 * M], BF, kind="ExternalInput")
    bm2_d = nc.dram_tensor("bm2", [M], F32, kind="ExternalInput")
    wm3_d = nc.dram_tensor("Wm3", [128, 2 * E], BF, kind="ExternalInput")
    bm3_d = nc.dram_tensor("bm3", [E], F32, kind="ExternalInput")
    eff_d = nc.dram_tensor("eff", [E], F32, kind="ExternalInput")
    out_d = nc.dram_tensor("out", [S, H], BF, kind="ExternalOutput")

    with tile.TileContext(nc) as tc:
        with (
            tc.tile_pool(name="persist", bufs=1) as pp,
            tc.tile_pool(name="router", bufs=1) as rp,
            tc.tile_pool(name="xt", bufs=1) as xtp,
            tc.tile_pool(name="w1", bufs=1) as w1p,
            tc.tile_pool(name="w2", bufs=1) as w2p,
            tc.tile_pool(name="bias", bufs=2) as bp,
            tc.tile_pool(name="he", bufs=2) as hep,
            tc.tile_pool(name="acc", bufs=1) as accp,
            tc.tile_pool(name="ye", bufs=2) as yep,
            tc.tile_pool(name="outb", bufs=2) as outp,
            tc.tile_pool(name="ps1", bufs=2, space=bass.MemorySpace.PSUM) as ps1p,
            tc.tile_pool(name="ps2", bufs=2, space=bass.MemorySpace.PSUM) as ps2p,
        ):
            # wbc layout: col 0 = w0, cols 1..7 = w_e/w_0, col 8 = w_7
            wbc = pp.tile([128, E + 1], F32)
            ones_bf = pp.tile([1, 128], BF)    # ones row for bias2 matmuls
            nc.vector.memset(ones_bf[:], 1.0)
            pooled_f = pp.tile([128, NKH], F32)
            nc.vector.memset(pooled_f[:], 0.0)

            # ---- PE warm-up: dummy matmuls on zeroed tiles ----
            dum_w = pp.tile([128, 128], BF)
            dum_x = pp.tile([128, 512], BF)
            nc.vector.memset(dum_w[:], 0.0)
            nc.vector.memset(dum_x[:], 0.0)
            dum_ps = ps1p.tile([128, 2, CHUNK], F32, tag="ps1", name="dum_ps")
            for i in range(NDUMMY):
                nc.tensor.matmul(
                    dum_ps[:, 0, :], dum_w[:], dum_x[:],
                    start=True, stop=True, skip_group_check=True,
                )

            # ---- all of x, transposed, resident for the whole kernel ----
            # ck0 rides the otherwise-idle Sync HWDGE queue in parallel with
            # expert 0's weights; ck1-3 (not needed until ~50us) queue on the
            # SWDGE FIFO *behind* W1/W2-e0 so they don't steal early HBM BW.
            xt_all = []
            for ck in range(NCHUNK):
                xt = xtp.tile([128, NKH, CHUNK], BF, tag=f"xt{ck}", name=f"xt{ck}")
                xt_all.append(xt)

            def load_w1(ck, e, engines=None):
                halves = []
                for half in range(2):
                    t = w1p.tile([128, 8, Hh], BF, tag=f"w1{half}",
                                 name=f"w1_{ck}_{e}_{half}")
                    eng = engines[half] if engines else nc.sync
                    eng.dma_start(t[:], w1_d[e, half])
                    halves.append(t)
                return halves

            def load_w2(ck, e):
                w2 = w2p.tile([128, 4, NFT, 512], BF, tag="w2", name=f"w2_{ck}_{e}")
                nc.sync.dma_start(w2[:], w2_d[e])
                return w2

            def load_b(ck, e):
                b1t = None
                if with_bias1:
                    b1t = bp.tile([128, NFT], F32, tag="b1", name=f"b1_{ck}_{e}")
                    nc.sync.dma_start(
                        b1t[:], b1_d[e].rearrange("(t p) -> p t", p=128)
                    )
                b2t = None
                if with_bias2:
                    b2t = bp.tile([1, H], BF, tag="b2", name=f"b2_{ck}_{e}")
                    nc.sync.dma_start(b2t[:], b2_d[e:e + 1, :])
                return b1t, b2t

            # Bulk traffic rides the Sync HWDGE FIFO (ramps ~4us earlier than
            # SWDGE and needs no Q7 descriptor generation); the small strided
            # router-weight loads stay on the SWDGE queue out of the way.
            with tc.high_priority():
                # Single HWDGE FIFO, explicit startup order: x chunk 0 and
                # W1-e0 kt0-7 (the two-pass L1's pass-1 inputs) first, then
                # W1 kt8-15, W2, x chunks 1-3. Transfers serialize per ring,
                # so pass 1 starts at ~20us (4MB in) instead of ~27 (8MB).
                nc.sync.dma_start(xt_all[0][:], x_d[0])
                w1h0 = load_w1(0, 0)
                b0 = load_b(0, 0)
                preload = {(0, 0): (w1h0, load_w2(0, 0), b0)}
                for ck in range(1, NCHUNK):
                    nc.sync.dma_start(xt_all[ck][:], x_d[ck])
                wm1 = rp.tile([128, NKH, M], BF)
                nc.gpsimd.dma_start(wm1[:], wm1_d[:].rearrange("p (t f) -> p t f", f=M))
                bm1 = rp.tile([128, 2], F32)
                nc.gpsimd.dma_start(bm1[:], bm1_d[:].rearrange("(t p) -> p t", p=128))
                wm2 = rp.tile([128, 2, M], BF)
                nc.gpsimd.dma_start(wm2[:], wm2_d[:].rearrange("p (t f) -> p t f", f=M))
                bm2 = rp.tile([128, 2], F32)
                nc.gpsimd.dma_start(bm2[:], bm2_d[:].rearrange("(t p) -> p t", p=128))
                wm3 = rp.tile([128, 2, E], BF)
                nc.gpsimd.dma_start(wm3[:], wm3_d[:].rearrange("p (t f) -> p t f", f=E))
                bm3 = rp.tile([1, E], F32)
                nc.gpsimd.dma_start(bm3[:], bm3_d[:].rearrange("(a e) -> a e", a=1))
                eff = rp.tile([1, E], F32)
                nc.gpsimd.dma_start(eff[:], eff_d[:].rearrange("(a e) -> a e", a=1))

            # ---- router pooling: one DVE free-dim reduce per chunk ----
            for ck in range(NCHUNK):
                ptmp = rp.tile([128, NKH], F32, tag="ptmp", name=f"ptmp{ck}")
                nc.vector.tensor_reduce(
                    ptmp[:], xt_all[ck][:, :, :], mybir.AxisListType.X, ALU.add
                )
                nc.vector.tensor_tensor(
                    pooled_f[:], pooled_f[:], ptmp[:], ALU.add
                )

            rst = {}

            def emit_router_mlp():
                """pooled_f -> router MLP -> double softmax -> rrow
                [w0, r1..r7, w7]. Emitted between e0 and e1: the serial
                DVE/ACT softmax chain overlaps e1's L1 matmuls."""
                pooled = rp.tile([128, NKH], BF)
                nc.vector.tensor_scalar(pooled[:], pooled_f[:], 1.0 / S, None, ALU.mult)
                ones_f = rp.tile([1, 128], F32)
                nc.vector.memset(ones_f[:], 1.0)
                ones_b1 = rp.tile([1, 1], BF)
                nc.vector.memset(ones_b1[:], 1.0)

                h1t = rp.tile([128, 2], BF)
                for ft in range(2):
                    ps = ps2p.tile([128, E + 1], F32, tag="ps2", name=f"rps1_{ft}")
                    for kt in range(NKH):
                        nc.tensor.matmul(
                            ps[:, 0:1],
                            wm1[:, kt, ft * 128:(ft + 1) * 128],
                            pooled[:, kt:kt + 1],
                            start=(kt == 0), stop=(kt == NKH - 1),
                        )
                    nc.vector.tensor_scalar(
                        h1t[:, ft:ft + 1], ps[:, 0:1], bm1[:, ft:ft + 1], 0.0,
                        ALU.add, ALU.max,
                    )
                h2t = rp.tile([128, 2], BF)
                for ft in range(2):
                    ps = ps2p.tile([128, E + 1], F32, tag="ps2", name=f"rps2_{ft}")
                    for kt in range(2):
                        nc.tensor.matmul(
                            ps[:, 0:1],
                            wm2[:, kt, ft * 128:(ft + 1) * 128],
                            h1t[:, kt:kt + 1],
                            start=(kt == 0), stop=(kt == 1),
                        )
                    nc.vector.tensor_scalar(
                        h2t[:, ft:ft + 1], ps[:, 0:1], bm2[:, ft:ft + 1], 0.0,
                        ALU.add, ALU.max,
                    )
                bm3b = rp.tile([1, E], BF)
                nc.vector.tensor_copy(bm3b[:], bm3[:])
                psl = ps2p.tile([128, E + 1], F32, tag="ps2", name="rpsl")
                for kt in range(2):
                    nc.tensor.matmul(
                        psl[0:1, 0:E], h2t[:, kt:kt + 1], wm3[:, kt, :],
                        start=(kt == 0), stop=False,
                    )
                nc.tensor.matmul(
                    psl[0:1, 0:E], ones_b1[0:1, 0:1], bm3b[0:1, :],
                    start=False, stop=True,
                )
                logits = rp.tile([1, E], F32)
                nc.vector.tensor_copy(logits[:], psl[0:1, 0:E])

                probs = rp.tile([1, E], F32)
                _softmax_1x8(nc, rp, logits[:], probs[:], "sm1")
                wpre = rp.tile([1, E], F32)
                nc.vector.tensor_tensor(wpre[:], probs[:], eff[:], ALU.mult)
                wrow = rp.tile([1, E], F32)
                _softmax_1x8(nc, rp, wpre[:], wrow[:], "sm2")

                # rrow = [w0, w1/w0, ..., w7/w0, w7]
                rcp0 = rp.tile([1, 1], F32)
                nc.vector.reciprocal(rcp0[:], wrow[0:1, 0:1])
                rrow = rp.tile([1, E + 1], F32)
                nc.vector.tensor_scalar(
                    rrow[:, 0:E], wrow[:], rcp0[0:1, 0:1], None, ALU.mult
                )
                nc.vector.tensor_copy(rrow[:, 0:1], wrow[0:1, 0:1])
                nc.vector.tensor_copy(rrow[:, E:E + 1], wrow[0:1, E - 1:E])

                rst["rrow"] = rrow
                rst["ones_f"] = ones_f

            def emit_router_bcast():
                """Broadcast rrow to all 128 partitions. Emitted after e1's
                L1 so the chain behind rrow is long done."""
                psw = ps2p.tile([128, E + 1], F32, tag="ps2", name="rpsw")
                nc.tensor.matmul(
                    psw[:], rst["ones_f"][0:1, :], rst["rrow"][0:1, :],
                    start=True, stop=True,
                )
                nc.vector.tensor_copy(wbc[:], psw[:])

            # ---------------- experts ----------------
            for ck in range(NCHUNK):
                xt = xt_all[ck]
                acc_tiles = [
                    accp.tile([128, H], F32, tag=f"acc{st}", name=f"acc{ck}_{st}")
                    for st in range(NST)
                ]
                out_tiles = [None] * NST
                for e in range(E):
                    if ck == 0 and e == 1:
                        emit_router_mlp()
                    if (ck, e) in preload:
                        w1h, w2, (b1t, b2t) = preload[(ck, e)]
                    else:
                        w1h = load_w1(ck, e)
                        w2 = load_w2(ck, e)
                        b1t, b2t = load_b(ck, e)

                    he = hep.tile([128, NFT, CHUNK], BF, tag="he", name=f"he_{ck}_{e}")
                    if ck == 0 and e == 0:
                        # startup special: contract kt 0-7 (first W1 half to
                        # arrive) across all 4 wide tiles, then kt 8-15 —
                        # PE starts ~6us earlier than waiting for full W1.
                        ps_sp = []
                        for fp in range(NFT // 2):
                            pool = ps1p if fp < 2 else ps2p
                            tag = "ps1" if fp < 2 else "ps2"
                            ps_sp.append(pool.tile(
                                [128, 2, CHUNK], F32, tag=tag, name=f"ps1sp_{fp}"
                            ))
                        for khalf in range(2):
                            for fp in range(NFT // 2):
                                for sub in range(2):
                                    ft = 2 * fp + sub
                                    for kt in range(khalf * 8, khalf * 8 + 8):
                                        nc.tensor.matmul(
                                            ps_sp[fp][:, sub, :],
                                            w1h[khalf][:, kt % 8,
                                                       ft * 128:(ft + 1) * 128],
                                            xt[:, kt, :],
                                            start=(kt == 0), stop=(kt == NKH - 1),
                                            skip_group_check=True,
                                        )
                        for fp in range(NFT // 2):
                            if with_bias1:
                                for sub in range(2):
                                    ft = 2 * fp + sub
                                    nc.scalar.activation(
                                        he[:, ft, :], ps_sp[fp][:, sub, :],
                                        AF.Relu, bias=b1t[:, ft:ft + 1],
                                    )
                            else:
                                nc.scalar.activation(
                                    he[:, 2 * fp:2 * fp + 2, :], ps_sp[fp][:], AF.Relu
                                )
                    for fp in range(NFT // 2):
                        if ck == 0 and e == 0:
                            break
                        ps = ps1p.tile([128, 2, CHUNK], F32, tag="ps1",
                                       name=f"ps1_{ck}_{e}_{fp}")
                        for sub in range(2):
                            ft = 2 * fp + sub
                            for kt in range(NKH):
                                nc.tensor.matmul(
                                    ps[:, sub, :],
                                    w1h[kt // 8][:, kt % 8, ft * 128:(ft + 1) * 128],
                                    xt[:, kt, :],
                                    start=(kt == 0), stop=(kt == NKH - 1),
                                )
                        if with_bias1:
                            for sub in range(2):
                                ft = 2 * fp + sub
                                nc.scalar.activation(
                                    he[:, ft, :], ps[:, sub, :], AF.Relu,
                                    bias=b1t[:, ft:ft + 1],
                                )
                        else:
                            nc.scalar.activation(
                                he[:, 2 * fp:2 * fp + 2, :], ps[:], AF.Relu
                            )

                    if ck == 0 and e == 1:
                        emit_router_bcast()

                    for st in range(NST):
                        if e == E - 1 and out_tiles[st] is None:
                            out_tiles[st] = outp.tile(
                                [128, H], BF, tag="outt", name=f"out_{ck}_{st}"
                            )
                        for hp in range(NHT // 2):
                            ps2 = ps2p.tile([128, 2, 512], F32, tag="ps2",
                                            name=f"ps2_{ck}_{e}_{st}_{hp}")
                            for sub in range(2):
                                ht = 2 * hp + sub
                                for fk in range(NFT):
                                    nc.tensor.matmul(
                                        ps2[:, sub, :],
                                        he[:, fk, st * 128:(st + 1) * 128],
                                        w2[:, ht, fk, :],
                                        start=(fk == 0),
                                        stop=(not with_bias2 and fk == NFT - 1),
                                    )
                                if with_bias2:
                                    nc.tensor.matmul(
                                        ps2[:, sub, :], ones_bf[0:1, :],
                                        b2t[0:1, ht * 512:(ht + 1) * 512],
                                        start=False, stop=True,
                                    )
                            accs = acc_tiles[st][:, hp * 1024:(hp + 1) * 1024]
                            if e == 0:
                                # no router dependency: plain tanh into acc
                                nc.scalar.activation(accs, ps2[:], AF.Tanh)
                            else:
                                ye = yep.tile([128, 1024], F32, tag="ye",
                                              name=f"ye_{ck}_{e}_{st}_{hp}")
                                nc.scalar.activation(ye[:], ps2[:], AF.Tanh)
                                if e < E - 1:
                                    nc.vector.scalar_tensor_tensor(
                                        accs, ye[:], wbc[:, e:e + 1], accs,
                                        ALU.mult, ALU.add,
                                    )
                                    if e == E - 2:
                                        # pre-scale by w0 off the critical path
                                        nc.vector.tensor_scalar(
                                            accs, accs, wbc[:, 0:1], None, ALU.mult
                                        )
                                else:
                                    # final: out = w7*tanh(z) + acc  (bf16)
                                    nc.vector.scalar_tensor_tensor(
                                        out_tiles[st][:, hp * 1024:(hp + 1) * 1024],
                                        ye[:], wbc[:, E:E + 1], accs,
                                        ALU.mult, ALU.add,
                                    )
                                    r0 = ck * CHUNK + st * 128
                                    nc.sync.dma_start(
                                        out_d[r0:r0 + 128,
                                              hp * 1024:(hp + 1) * 1024],
                                        out_tiles[st][:, hp * 1024:(hp + 1) * 1024],
                                    )

    nc.compile()
    return nc


def _get_nc(with_bias1=False, with_bias2=False):
    key = (with_bias1, with_bias2)
    if key not in _NC:
        _NC[key] = build(with_bias1, with_bias2)
    return _NC[key]


def prep_in_maps(inputs):
    x = np.asarray(inputs["x"], np.float32)
    xbf = x.astype(BF16)
    # pre-transpose to [ht, ck, p, c] blocks (see build()): xT[h, s] blocked
    xts = [
        np.ascontiguousarray(
            xbf[b].T.reshape(NKH, 128, NCHUNK, CHUNK).transpose(2, 1, 0, 3)
        )
        for b in range(B)
    ]
    w1 = np.asarray(inputs["W1"], np.float32).astype(BF16)   # [E, H, Hh]
    w2 = np.asarray(inputs["W2"], np.float32).astype(BF16)   # [E, Hh, H]
    # shuffle to SBUF layout (see build()): halves x partition-major
    w1s = np.ascontiguousarray(
        w1.reshape(E, 2, 8, 128, Hh).transpose(0, 1, 3, 2, 4)
    )
    w2s = np.ascontiguousarray(
        w2.reshape(E, 8, 128, 4, 512).transpose(0, 2, 3, 1, 4)
    )
    wm1 = np.asarray(inputs["Wm1"], np.float32).astype(BF16)
    wm1s = np.ascontiguousarray(
        wm1.reshape(16, 128, M).transpose(1, 0, 2).reshape(128, 16 * M)
    )
    wm2 = np.asarray(inputs["Wm2"], np.float32).astype(BF16)
    wm2s = np.ascontiguousarray(
        wm2.reshape(2, 128, M).transpose(1, 0, 2).reshape(128, 2 * M)
    )
    wm3 = np.asarray(inputs["Wm3"], np.float32).astype(BF16)
    wm3s = np.ascontiguousarray(
        wm3.reshape(2, 128, E).transpose(1, 0, 2).reshape(128, 2 * E)
    )
    shared = {
        "W1": w1s,
        "W2": w2s,
        "b1": np.asarray(inputs["b1"], np.float32),
        "b2": np.asarray(inputs["b2"], np.float32).astype(BF16),
        "Wm1": wm1s,
        "bm1": np.asarray(inputs["bm1"], np.float32),
        "Wm2": wm2s,
        "bm2": np.asarray(inputs["bm2"], np.float32),
        "Wm3": wm3s,
        "bm3": np.asarray(inputs["bm3"], np.float32),
        "eff": np.asarray(inputs["eff"], np.float32),
    }
    return [dict(shared, x=xts[b]) for b in range(B)]


def kernel(**inputs):
    wb1 = bool(np.any(np.asarray(inputs["b1"])))
    wb2 = bool(np.any(np.asarray(inputs["b2"])))
    nc = _get_nc(wb1, wb2)
    in_maps = prep_in_maps(inputs)
    res = run_bass_kernel_spmd(nc, in_maps, core_ids=list(range(B)))
    return np.stack([np.asarray(r["out"]).astype(np.float32) for r in res.results])


if __name__ == "__main__":
    rng = np.random.default_rng(0)
    s = 0.02
    ins = {
        "x": rng.standard_normal((B, S, H), dtype=np.float32),
        "Wm1": rng.standard_normal((H, M), dtype=np.float32) * s,
        "bm1": np.zeros(M, np.float32),
        "Wm2": rng.standard_normal((M, M), dtype=np.float32) * s,
        "bm2": np.zeros(M, np.float32),
        "Wm3": rng.standard_normal((M, E), dtype=np.float32) * s,
        "bm3": np.zeros(E, np.float32),
        "W1": rng.standard_normal((E, H, Hh), dtype=np.float32) * s,
        "b1": np.zeros((E, Hh), np.float32),
        "W2": rng.standard_normal((E, Hh, H), dtype=np.float32) * s,
        "b2": np.zeros((E, H), np.float32),
        "eff": np.ones(E, np.float32),
    }
    out = kernel(**ins)
    print("out", out.shape, out.dtype, float(np.abs(out).mean()))


# revision 24
# speedup vs baseline: 1.0036x; 1.0008x over previous
"""MetacognitionModule (MoE routing) Trainium2 kernel.

Sharding: data-parallel over batch — core i handles batch i (B=8, 8 cores).
Everything is local per core: the router (mean-pool -> 3-layer MLP -> double
softmax) and all 8 expert MLPs run on the core that owns the batch, so no
collectives are needed.

Per-core dataflow (S=2048 tokens, H=2048, Hh=1024, E=8 experts):
  - x arrives host-pretransposed and blocked; all 4 chunks' xT tiles load as
    single 2MiB contiguous DMAs and stay resident (x read from HBM once).
    Bulk traffic (x, W1/W2 streams, out stores) rides the Sync HWDGE FIFO in
    an explicit startup order; small strided router-weight loads ride SWDGE.
  - PE warm-up: dummy matmuls on memset tiles head the PE stream, keeping the
    HAM clock-gate at 8/8 while the first weights/x DMAs land (~25us, HBM
    ramps from idle).
  - Expert 0 chunk 0 runs L1 as two kt-half passes so compute starts on the
    first half of W1 (pass tiles borrow both PSUM pools).
  - Router: pooled = mean_s x via one DVE free-dim reduce per chunk. The MLP
    + double softmax are emitted between e0 and e1 (the serial DVE/ACT chain
    overlaps e1's L1); the broadcast matmul lands after e1's L1. wbc holds
    [w0, w1/w0 .. w7/w0, w7] so expert 0's combine needs no router output.
  - Experts, chunked over S (4 chunks of 512 tokens), expert-inner, weights
    streamed per (chunk, expert), wide 2-bank PSUM tiles halve sync traffic:
      L1: heT[f,s] = relu(W1[e].T @ xT)        (+b1 via ACT if nonzero)
      L2: z[s,h]  = heT.T @ W2[e]              (+b2 via ones-matmul if nonzero)
      e=0   : acc       = tanh(z)              (ACT straight to SBUF acc)
      e=1..6: acc      += (w_e/w_0) * tanh(z)  (ACT tanh + DVE fused mul-add)
      after e6: acc    *= w_0                  (off critical path)
      e=7   : out_bf16  = w_7 * tanh(z) + acc  (DVE writes bf16 directly)
  - out stored per (s-subtile, h-half) as results finish, natural [S,H]
    layout, bf16 (host casts back to f32).
All expert matmuls bf16 with fp32 PSUM accumulation.
"""

import sys

for _p in ("/opt/trn_rl_repo", "/root/.axon_site/_ro/trn_rl_repo"):
    if _p not in sys.path:
        sys.path.insert(0, _p)

import ml_dtypes
import numpy as np

import concourse.bacc as bacc
import concourse.bass as bass
import concourse.mybir as mybir
import concourse.tile as tile
from concourse.bass_utils import run_bass_kernel_spmd

BF16 = ml_dtypes.bfloat16
F32 = mybir.dt.float32
BF = mybir.dt.bfloat16
AF = mybir.ActivationFunctionType
ALU = mybir.AluOpType

B, S, H, M, E = 8, 2048, 2048, 256, 8
Hh = H // 2
CHUNK = 512
NCHUNK = S // CHUNK          # 4
NST = CHUNK // 128           # 4 s-subtiles per chunk
NHT = H // 512               # 4 output h tiles (512 wide)
NFT = Hh // 128              # 8 L1 output f tiles
NKH = H // 128               # 16 k tiles over h
NDUMMY = 72                  # PE warm-up matmuls: bridge to first weights (~25us)

_NC = {}


def _softmax_1x8(nc, pool, vec, out, tagp):
    """vec, out: [1, E] f32 sbuf APs. out = softmax(vec) along free dim.
    No max-subtraction: inputs here are probabilities or ~1e-3 logits, so
    exp() is always in range."""
    t = pool.tile([1, E], F32, tag=tagp + "t", name=tagp + "t")
    nc.scalar.activation(t[:], vec, AF.Exp)
    sm = pool.tile([1, 1], F32, tag=tagp + "sm", name=tagp + "sm")
    nc.vector.tensor_reduce(sm[:], t[:], mybir.AxisListType.X, ALU.add)
    rs = pool.tile([1, 1], F32, tag=tagp + "rs", name=tagp + "rs")
    nc.vector.reciprocal(rs[:], sm[:])
    nc.vector.tensor_scalar(out, t[:], rs[0:1, 0:1], None, ALU.mult)


def build(with_bias1=False, with_bias2=False):
    nc = bacc.Bacc("TRN2", target_bir_lowering=False, debug=False, num_devices=B)

    # x arrives host-pretransposed: [ck, p, ht, c] so each chunk's xT tile is
    # ONE contiguous 2MiB full-rate DMA (HWDGE completion latency amortized).
    x_d = nc.dram_tensor("x", [NCHUNK, 128, NKH, CHUNK], BF, kind="ExternalInput")
    # W1/W2 arrive host-preshuffled to SBUF layout:
    # W1: [E, half, p, kt, f]  (host-preshuffled, halves of h-contraction)
    # W2: [E, p, ht, fk, c]    (host-preshuffled, ht-major)
    w1_d = nc.dram_tensor("W1", [E, 2, 128, 8, Hh], BF, kind="ExternalInput")
    w2_d = nc.dram_tensor("W2", [E, 128, 4, NFT, 512], BF, kind="ExternalInput")
    b1_d = nc.dram_tensor("b1", [E, Hh], F32, kind="ExternalInput")
    b2_d = nc.dram_tensor("b2", [E, H], BF, kind="ExternalInput")
    wm1_d = nc.dram_tensor("Wm1", [128, NKH * M], BF, kind="ExternalInput")
    bm1_d = nc.dram_tensor("bm1", [M], F32, kind="ExternalInput")
    wm2_d = nc.dram_tensor("Wm2", [128, 2 * M], BF, kind="ExternalInput")
    bm2_d = nc.dram_tensor("bm2", [M], F32, kind="ExternalInput")
    wm3_d = nc.dram_tensor("Wm3", [128, 2 * E], BF, kind="ExternalInput")
    bm3_d = nc.dram_tensor("bm3", [E], F32, kind="ExternalInput")
    eff_d = nc.dram_tensor("eff", [E], F32, kind="ExternalInput")
    out_d = nc.dram_tensor("out", [S, H], BF, kind="ExternalOutput")

    with tile.TileContext(nc) as tc:
        with (
            tc.tile_pool(name="persist", bufs=1) as pp,
            tc.tile_pool(name="router", bufs=1) as rp,
            tc.tile_pool(name="xt", bufs=1) as xtp,
            tc.tile_pool(name="w1", bufs=1) as w1p,
            tc.tile_pool(name="w2", bufs=1) as w2p,
            tc.tile_pool(name="bias", bufs=2) as bp,
            tc.tile_pool(name="he", bufs=2) as hep,
            tc.tile_pool(name="acc", bufs=1) as accp,
            tc.tile_pool(name="ye", bufs=2) as yep,
            tc.tile_pool(name="outb", bufs=2) as outp,
            tc.tile_pool(name="ps1", bufs=2, space=bass.MemorySpace.PSUM) as ps1p,
            tc.tile_pool(name="ps2", bufs=2, space=bass.MemorySpace.PSUM) as ps2p,
        ):
            # wbc layout: col 0 = w0, cols 1..7 = w_e/w_0, col 8 = w_7
            wbc = pp.tile([128, E + 1], F32)
            ones_bf = pp.tile([1, 128], BF)    # ones row for bias2 matmuls
            nc.vector.memset(ones_bf[:], 1.0)
            pooled_f = pp.tile([128, NKH], F32)
            nc.vector.memset(pooled_f[:], 0.0)

            # ---- PE warm-up: dummy matmuls on zeroed tiles ----
            dum_w = pp.tile([128, 128], BF)
            dum_x = pp.tile([128, 512], BF)
            nc.vector.memset(dum_w[:], 0.0)
            nc.vector.memset(dum_x[:], 0.0)
            dum_ps = ps1p.tile([128, 2, CHUNK], F32, tag="ps1", name="dum_ps")
            for i in range(NDUMMY):
                nc.tensor.matmul(
                    dum_ps[:, 0, :], dum_w[:], dum_x[:],
                    start=True, stop=True, skip_group_check=True,
                )

            # ---- all of x, transposed, resident for the whole kernel ----
            # ck0 rides the otherwise-idle Sync HWDGE queue in parallel with
            # expert 0's weights; ck1-3 (not needed until ~50us) queue on the
            # SWDGE FIFO *behind* W1/W2-e0 so they don't steal early HBM BW.
            xt_all = []
            for ck in range(NCHUNK):
                xt = xtp.tile([128, NKH, CHUNK], BF, tag=f"xt{ck}", name=f"xt{ck}")
                xt_all.append(xt)

            def load_w1(ck, e, engines=None):
                halves = []
                for half in range(2):
                    t = w1p.tile([128, 8, Hh], BF, tag=f"w1{half}",
                                 name=f"w1_{ck}_{e}_{half}")
                    eng = engines[half] if engines else nc.sync
                    eng.dma_start(t[:], w1_d[e, half])
                    halves.append(t)
                return halves

            def load_w2(ck, e):
                w2 = w2p.tile([128, 4, NFT, 512], BF, tag="w2", name=f"w2_{ck}_{e}")
                nc.sync.dma_start(w2[:], w2_d[e])
                return w2

            def load_b(ck, e):
                b1t = None
                if with_bias1:
                    b1t = bp.tile([128, NFT], F32, tag="b1", name=f"b1_{ck}_{e}")
                    nc.sync.dma_start(
                        b1t[:], b1_d[e].rearrange("(t p) -> p t", p=128)
                    )
                b2t = None
                if with_bias2:
                    b2t = bp.tile([1, H], BF, tag="b2", name=f"b2_{ck}_{e}")
                    nc.sync.dma_start(b2t[:], b2_d[e:e + 1, :])
                return b1t, b2t

            # Bulk traffic rides the Sync HWDGE FIFO (ramps ~4us earlier than
            # SWDGE and needs no Q7 descriptor generation); the small strided
            # router-weight loads stay on the SWDGE queue out of the way.
            with tc.high_priority():
                # Single HWDGE FIFO, explicit startup order: x chunk 0 and
                # W1-e0 kt0-7 (the two-pass L1's pass-1 inputs) first, then
                # W1 kt8-15, W2, x chunks 1-3. Transfers serialize per ring,
                # so pass 1 starts at ~20us (4MB in) instead of ~27 (8MB).
                nc.sync.dma_start(xt_all[0][:], x_d[0])
                w1h0 = load_w1(0, 0)
                b0 = load_b(0, 0)
                preload = {(0, 0): (w1h0, load_w2(0, 0), b0)}
                for ck in range(1, NCHUNK):
                    nc.sync.dma_start(xt_all[ck][:], x_d[ck])
                wm1 = rp.tile([128, NKH, M], BF)
                nc.gpsimd.dma_start(wm1[:], wm1_d[:].rearrange("p (t f) -> p t f", f=M))
                bm1 = rp.tile([128, 2], F32)
                nc.gpsimd.dma_start(bm1[:], bm1_d[:].rearrange("(t p) -> p t", p=128))
                wm2 = rp.tile([128, 2, M], BF)
                nc.gpsimd.dma_start(wm2[:], wm2_d[:].rearrange("p (t f) -> p t f", f=M))
                bm2 = rp.tile([128, 2], F32)
                nc.gpsimd.dma_start(bm2[:], bm2_d[:].rearrange("(t p) -> p t", p=128))
                wm3 = rp.tile([128, 2, E], BF)
                nc.gpsimd.dma_start(wm3[:], wm3_d[:].rearrange("p (t f) -> p t f", f=E))
                bm3 = rp.tile([1, E], F32)
                nc.gpsimd.dma_start(bm3[:], bm3_d[:].rearrange("(a e) -> a e", a=1))
                eff = rp.tile([1, E], F32)
                nc.gpsimd.dma_start(eff[:], eff_d[:].rearrange("(a e) -> a e", a=1))

            # ---- router pooling: one DVE free-dim reduce per chunk ----
            for ck in range(NCHUNK):
                ptmp = rp.tile([128, NKH], F32, tag="ptmp", name=f"ptmp{ck}")
                nc.vector.tensor_reduce(
                    ptmp[:], xt_all[ck][:, :, :], mybir.AxisListType.X, ALU.add
                )
                nc.vector.tensor_tensor(
                    pooled_f[:], pooled_f[:], ptmp[:], ALU.add
                )

            rst = {}

            def emit_router_mlp():
                """pooled_f -> router MLP -> double softmax -> rrow
                [w0, r1..r7, w7]. Emitted between e0 and e1: the serial
                DVE/ACT softmax chain overlaps e1's L1 matmuls."""
                pooled = rp.tile([128, NKH], BF)
                nc.vector.tensor_scalar(pooled[:], pooled_f[:], 1.0 / S, None, ALU.mult)
                ones_f = rp.tile([1, 128], F32)
                nc.vector.memset(ones_f[:], 1.0)
                ones_b1 = rp.tile([1, 1], BF)
                nc.vector.memset(ones_b1[:], 1.0)

                h1t = rp.tile([128, 2], BF)
                for ft in range(2):
                    ps = ps2p.tile([128, E + 1], F32, tag="ps2", name=f"rps1_{ft}")
                    for kt in range(NKH):
                        nc.tensor.matmul(
                            ps[:, 0:1],
                            wm1[:, kt, ft * 128:(ft + 1) * 128],
                            pooled[:, kt:kt + 1],
                            start=(kt == 0), stop=(kt == NKH - 1),
                        )
                    nc.vector.tensor_scalar(
                        h1t[:, ft:ft + 1], ps[:, 0:1], bm1[:, ft:ft + 1], 0.0,
                        ALU.add, ALU.max,
                    )
                h2t = rp.tile([128, 2], BF)
                for ft in range(2):
                    ps = ps2p.tile([128, E + 1], F32, tag="ps2", name=f"rps2_{ft}")
                    for kt in range(2):
                        nc.tensor.matmul(
                            ps[:, 0:1],
                            wm2[:, kt, ft * 128:(ft + 1) * 128],
                            h1t[:, kt:kt + 1],
                            start=(kt == 0), stop=(kt == 1),
                        )
                    nc.vector.tensor_scalar(
                        h2t[:, ft:ft + 1], ps[:, 0:1], bm2[:, ft:ft + 1], 0.0,
                        ALU.add, ALU.max,
                    )
                bm3b = rp.tile([1, E], BF)
                nc.vector.tensor_copy(bm3b[:], bm3[:])
                psl = ps2p.tile([128, E + 1], F32, tag="ps2", name="rpsl")
                for kt in range(2):
                    nc.tensor.matmul(
                        psl[0:1, 0:E], h2t[:, kt:kt + 1], wm3[:, kt, :],
                        start=(kt == 0), stop=False,
                    )
                nc.tensor.matmul(
                    psl[0:1, 0:E], ones_b1[0:1, 0:1], bm3b[0:1, :],
                    start=False, stop=True,
                )
                logits = rp.tile([1, E], F32)
                nc.vector.tensor_copy(logits[:], psl[0:1, 0:E])

                probs = rp.tile([1, E], F32)
                _softmax_1x8(nc, rp, logits[:], probs[:], "sm1")
                wpre = rp.tile([1, E], F32)
                nc.vector.tensor_tensor(wpre[:], probs[:], eff[:], ALU.mult)
                wrow = rp.tile([1, E], F32)
                _softmax_1x8(nc, rp, wpre[:], wrow[:], "sm2")

                # rrow = [w0, w1/w0, ..., w7/w0, w7]
                rcp0 = rp.tile([1, 1], F32)
                nc.vector.reciprocal(rcp0[:], wrow[0:1, 0:1])
                rrow = rp.tile([1, E + 1], F32)
                nc.vector.tensor_scalar(
                    rrow[:, 0:E], wrow[:], rcp0[0:1, 0:1], None, ALU.mult
                )
                nc.vector.tensor_copy(rrow[:, 0:1], wrow[0:1, 0:1])
                nc.vector.tensor_copy(rrow[:, E:E + 1], wrow[0:1, E - 1:E])

                rst["rrow"] = rrow
                rst["ones_f"] = ones_f

            def emit_router_bcast():
                """Broadcast rrow to all 128 partitions. Emitted after e1's
                L1 so the chain behind rrow is long done."""
                psw = ps2p.tile([128, E + 1], F32, tag="ps2", name="rpsw")
                nc.tensor.matmul(
                    psw[:], rst["ones_f"][0:1, :], rst["rrow"][0:1, :],
                    start=True, stop=True,
                )
                nc.vector.tensor_copy(wbc[:], psw[:])

            # ---------------- experts ----------------
            for ck in range(NCHUNK):
                xt = xt_all[ck]
                acc_tiles = [
                    accp.tile([128, H], F32, tag=f"acc{st}", name=f"acc{ck}_{st}")
                    for st in range(NST)
                ]
                out_tiles = [None] * NST
                for e in range(E):
                    if ck == 0 and e == 1:
                        emit_router_mlp()
                    if (ck, e) in preload:
                        w1h, w2, (b1t, b2t) = preload[(ck, e)]
                    else:
                        w1h = load_w1(ck, e)
                        w2 = load_w2(ck, e)
                        b1t, b2t = load_b(ck, e)

                    he = hep.tile([128, NFT, CHUNK], BF, tag="he", name=f"he_{ck}_{e}")
                    if ck == 0 and e == 0:
                        # startup special: contract kt 0-7 (first W1 half to
                        # arrive) across all 4 wide tiles, then kt 8-15 —
                        # PE starts ~6us earlier than waiting for full W1.
                        ps_sp = []
                        for fp in range(NFT // 2):
                            pool = ps1p if fp < 2 else ps2p
                            tag = "ps1" if fp < 2 else "ps2"
                            ps_sp.append(pool.tile(
                                [128, 2, CHUNK], F32, tag=tag, name=f"ps1sp_{fp}"
                            ))
                        for khalf in range(2):
                            for fp in range(NFT // 2):
                                for sub in range(2):
                                    ft = 2 * fp + sub
                                    for kt in range(khalf * 8, khalf * 8 + 8):
                                        nc.tensor.matmul(
                                            ps_sp[fp][:, sub, :],
                                            w1h[khalf][:, kt % 8,
                                                       ft * 128:(ft + 1) * 128],
                                            xt[:, kt, :],
                                            start=(kt == 0), stop=(kt == NKH - 1),
                                            skip_group_check=True,
                                        )
                        for fp in range(NFT // 2):
                            if with_bias1:
                                for sub in range(2):
                                    ft = 2 * fp + sub
                                    nc.scalar.activation(
                                        he[:, ft, :], ps_sp[fp][:, sub, :],
                                        AF.Relu, bias=b1t[:, ft:ft + 1],
                                    )
                            else:
                                nc.scalar.activation(
                                    he[:, 2 * fp:2 * fp + 2, :], ps_sp[fp][:], AF.Relu
                                )
                    for fp in range(NFT // 2):
                        if ck == 0 and e == 0:
                            break
                        ps = ps1p.tile([128, 2, CHUNK], F32, tag="ps1",
                                       name=f"ps1_{ck}_{e}_{fp}")
                        for sub in range(2):
                            ft = 2 * fp + sub
                            for kt in range(NKH):
                                nc.tensor.matmul(
                                    ps[:, sub, :],
                                    w1h[kt // 8][:, kt % 8, ft * 128:(ft + 1) * 128],
                                    xt[:, kt, :],
                                    start=(kt == 0), stop=(kt == NKH - 1),
                                )
                        if with_bias1:
                            for sub in range(2):
                                ft = 2 * fp + sub
                                nc.scalar.activation(
                                    he[:, ft, :], ps[:, sub, :], AF.Relu,
                                    bias=b1t[:, ft:ft + 1],
                                )
                        else:
                            nc.scalar.activation(
                                he[:, 2 * fp:2 * fp + 2, :], ps[:], AF.Relu
                            )

                    if ck == 0 and e == 1:
                        emit_router_bcast()

                    for st in range(NST):
                        if e == E - 1 and out_tiles[st] is None:
                            out_tiles[st] = outp.tile(
                                [128, H], BF, tag="outt", name=f"out_{ck}_{st}"
                            )
                        for hp in range(NHT // 2):
                            ps2 = ps2p.tile([128, 2, 512], F32, tag="ps2",
                                            name=f"ps2_{ck}_{e}_{st}_{hp}")
                            for sub in range(2):
                                ht = 2 * hp + sub
                                for fk in range(NFT):
                                    nc.tensor.matmul(
                                        ps2[:, sub, :],
                                        he[:, fk, st * 128:(st + 1) * 128],
                                        w2[:, ht, fk, :],
                                        start=(fk == 0),
                                        stop=(not with_bias2 and fk == NFT - 1),
                                    )
                                if with_bias2:
                                    nc.tensor.matmul(
                                        ps2[:, sub, :], ones_bf[0:1, :],
                                        b2t[0:1, ht * 512:(ht + 1) * 512],
                                        start=False, stop=True,
                                    )
                            accs = acc_tiles[st][:, hp * 1024:(hp + 1) * 1024]
                            if e == 0:
                                # no router dependency: plain tanh into acc
                                nc.scalar.activation(accs, ps2[:], AF.Tanh)
                            else:
                                ye = yep.tile([128, 1024], F32, tag="ye",
                                              name=f"ye_{ck}_{e}_{st}_{hp}")
                                nc.scalar.activation(ye[:], ps2[:], AF.Tanh)
                                if e < E - 1:
                                    nc.vector.scalar_tensor_tensor(
                                        accs, ye[:], wbc[:, e:e + 1], accs,
                                        ALU.mult, ALU.add,
                                    )
                                    if e == E - 2:
                                        # pre-scale by w0 off the critical path
                                        nc.vector.tensor_scalar(
                                            accs, accs, wbc[:, 0:1], None, ALU.mult
                                        )
                                else:
                                    # final: out = w7*tanh(z) + acc  (bf16)
                                    nc.vector.scalar_tensor_tensor(
                                        out_tiles[st][:, hp * 1024:(hp + 1) * 1024],
                                        ye[:], wbc[:, E:E + 1], accs,
                                        ALU.mult, ALU.add,
                                    )
                                    r0 = ck * CHUNK + st * 128
                                    nc.sync.dma_start(
                                        out_d[r0:r0 + 128,
                                              hp * 1024:(hp + 1) * 1024],
                                        out_tiles[st][:, hp * 1024:(hp + 1) * 1024],
                                    )

    nc.compile()
    return nc


def _get_nc(with_bias1=False, with_bias2=False):
    key = (with_bias1, with_bias2)
    if key not in _NC:
        _NC[key] = build(with_bias1, with_bias2)
    return _NC[key]


def prep_in_maps(inputs):
    x = np.asarray(inputs["x"], np.float32)
    xbf = x.astype(BF16)
    # pre-transpose to [ht, ck, p, c] blocks (see build()): xT[h, s] blocked
    xts = [
        np.ascontiguousarray(
            xbf[b].T.reshape(NKH, 128, NCHUNK, CHUNK).transpose(2, 1, 0, 3)
        )
        for b in range(B)
    ]
    w1 = np.asarray(inputs["W1"], np.float32).astype(BF16)   # [E, H, Hh]
    w2 = np.asarray(inputs["W2"], np.float32).astype(BF16)   # [E, Hh, H]
    # shuffle to SBUF layout (see build()): halves x partition-major
    w1s = np.ascontiguousarray(
        w1.reshape(E, 2, 8, 128, Hh).transpose(0, 1, 3, 2, 4)
    )
    w2s = np.ascontiguousarray(
        w2.reshape(E, 8, 128, 4, 512).transpose(0, 2, 3, 1, 4)
    )
    wm1 = np.asarray(inputs["Wm1"], np.float32).astype(BF16)
    wm1s = np.ascontiguousarray(
        wm1.reshape(16, 128, M).transpose(1, 0, 2).reshape(128, 16 * M)
    )
    wm2 = np.asarray(inputs["Wm2"], np.float32).astype(BF16)
    wm2s = np.ascontiguousarray(
        wm2.reshape(2, 128, M).transpose(1, 0, 2).reshape(128, 2 * M)
    )
    wm3 = np.asarray(inputs["Wm3"], np.float32).astype(BF16)
    wm3s = np.ascontiguousarray(
        wm3.reshape(2, 128, E).transpose(1, 0, 2).reshape(128, 2 * E)
    )
    shared = {
        "W1": w1s,
        "W2": w2s,
        "b1": np.asarray(inputs["b1"], np.float32),
        "b2": np.asarray(inputs["b2"], np.float32).astype(BF16),
        "Wm1": wm1s,
        "bm1": np.asarray(inputs["bm1"], np.float32),
        "Wm2": wm2s,
        "bm2": np.asarray(inputs["bm2"], np.float32),
        "Wm3": wm3s,
        "bm3": np.asarray(inputs["bm3"], np.float32),
        "eff": np.asarray(inputs["eff"], np.float32),
    }
    return [dict(shared, x=xts[b]) for b in range(B)]


def kernel(**inputs):
    wb1 = bool(np.any(np.asarray(inputs["b1"])))
    wb2 = bool(np.any(np.asarray(inputs["b2"])))
    nc = _get_nc(wb1, wb2)
    in_maps = prep_in_maps(inputs)
    res = run_bass_kernel_spmd(nc, in_maps, core_ids=list(range(B)))
    return np.stack([np.asarray(r["out"]).astype(np.float32) for r in res.results])


if __name__ == "__main__":
    rng = np.random.default_rng(0)
    s = 0.02
    ins = {
        "x": rng.standard_normal((B, S, H), dtype=np.float32),
        "Wm1": rng.standard_normal((H, M), dtype=np.float32) * s,
        "bm1": np.zeros(M, np.float32),
        "Wm2": rng.standard_normal((M, M), dtype=np.float32) * s,
        "bm2": np.zeros(M, np.float32),
        "Wm3": rng.standard_normal((M, E), dtype=np.float32) * s,
        "bm3": np.zeros(E, np.float32),
        "W1": rng.standard_normal((E, H, Hh), dtype=np.float32) * s,
        "b1": np.zeros((E, Hh), np.float32),
        "W2": rng.standard_normal((E, Hh, H), dtype=np.float32) * s,
        "b2": np.zeros((E, H), np.float32),
        "eff": np.ones(E, np.float32),
    }
    out = kernel(**ins)
    print("out", out.shape, out.dtype, float(np.abs(out).mean()))
